# revision 8
# baseline (speedup 1.0000x reference)
"""Trainium2 Bass kernel for ChamferLoss (B=8, C=3, N=4096), 8 NeuronCores.

Strategy: data-parallel over batch. Core b computes batch b fully:
  D[n,m] = ||x_n||^2 + ||y_m||^2 - 2 x_n.y_m   (x = ori, y = adv points)
  d1 = mean_n relu(min_m D),  d2 = mean_m relu(min_n D)
Host combines: mean_b max(d1_b, d2_b).

The -2*x.y matmul has contraction K=3; fp32 matmul is 4x slower on PE, so
each fp32 value v is split v = vh + vl (bf16 pair) and the product uses the
3-term expansion  x.y ~= xh.yh + xh.yl + xl.yh  (error ~2^-16 relative).
The squared norms are folded into the same matmul via constant-one rows, so
PSUM holds complete distance values.

Drain design (v3, single matmul pass):
 - ACT copies every [128,2048] PSUM unit into the two halves of a
   per-slab [128,4096] bf16 SBUF tile (ACT is the only engine besides DVE
   that can read PSUM; copies keep the PE->ACT pipeline free of DVE).
 - DVE works purely on bf16 SBUF at 2x rate: per slab one fold
   M = min(half0, half1) feeding a level-batched min-tree -> row minima
   (d1), plus one running accumulate CM = min(CM, Cslab) which preserves
   per-column minima across slabs.
 - Column direction (d2) finishes with PE transposes of CM (32 128x128
   blocks into PSUM) and two segmented tensor_reduce ops.
This reads each distance exactly once from PSUM and exactly twice in
bf16, with no second transposed matmul pass.
"""

import sys

sys.path.insert(0, "/opt/trn_rl_repo")

import numpy as np

import concourse.bass as bass  # noqa: F401  (registers engine types)
import concourse.masks as masks
import concourse.tile as tile
from concourse import bacc, bass_utils, mybir

B, C, N = 8, 3, 4096
NCORES = 8
NO = 32  # n_outer blocks of 128
NI = 128  # n_inner
F32 = mybir.dt.float32
BF16 = mybir.dt.bfloat16
MIN = mybir.AluOpType.min
K = 13  # contraction rows: 9 coord product terms + 2 sq rows + 2 one rows

_CACHE = {}


def _prep_pointset(nc, tc, sb, rr, v_dram):
    """Load [3, 4096] fp32 points; return dict of packed SBUF tiles."""
    vp = sb.tile([96, 128], F32)
    nc.sync.dma_start(vp[:], v_dram.rearrange("c (no ni) -> (c no) ni", ni=NI))

    vh = sb.tile([96, 128], BF16)
    nc.vector.tensor_copy(vh[:], vp[:])
    vl = sb.tile([96, 128], BF16)
    nc.vector.tensor_sub(vl[:], vp[:], vh[:])
    m2h = sb.tile([96, 128], BF16)
    nc.vector.tensor_scalar_mul(m2h[:], vh[:], -2.0)
    m2l = sb.tile([96, 128], BF16)
    nc.vector.tensor_scalar_mul(m2l[:], vl[:], -2.0)

    vsq = sb.tile([96, 128], F32)
    nc.vector.tensor_mul(vsq[:], vp[:], vp[:])
    vsqr = sb.tile([32, 384], F32)
    for c in range(3):
        rr.dma(vsqr[:, 128 * c : 128 * (c + 1)], vsq[32 * c : 32 * (c + 1), :])
    v2 = sb.tile([32, 128], F32)
    nc.vector.tensor_add(v2[:], vsqr[:, 0:128], vsqr[:, 128:256])
    nc.vector.tensor_add(v2[:], v2[:], vsqr[:, 256:384])
    v2h = sb.tile([32, 128], BF16)
    nc.vector.tensor_copy(v2h[:], v2[:])
    v2l = sb.tile([32, 128], BF16)
    nc.vector.tensor_sub(v2l[:], v2[:], v2h[:])
    return dict(vh=vh, vl=vl, m2h=m2h, m2l=m2l, v2h=v2h, v2l=v2l)


class _DmaRR:
    def __init__(self, nc):
        self.engines = [nc.sync, nc.scalar, nc.gpsimd]
        self.i = 0

    def dma(self, out, in_):
        e = self.engines[self.i % len(self.engines)]
        self.i += 1
        e.dma_start(out, in_)


def _assemble_lhs(nc, rr, sb, p, ones64, name):
    m = sb.tile([128, N], BF16, name=name)
    rr.dma(m[0:3, :], p["m2h"][:])
    rr.dma(m[3:6, :], p["m2h"][:])
    rr.dma(m[6:9, :], p["m2l"][:])
    rr.dma(m[9:11, :], ones64[:])
    rr.dma(m[11:12, :], p["v2h"][:])
    rr.dma(m[12:13, :], p["v2l"][:])
    return m


def _assemble_rhs(nc, rr, sb, p, ones64, name):
    m = sb.tile([128, N], BF16, name=name)
    rr.dma(m[0:3, :], p["vh"][:])
    rr.dma(m[3:6, :], p["vl"][:])
    rr.dma(m[6:9, :], p["vh"][:])
    rr.dma(m[9:10, :], p["v2h"][:])
    rr.dma(m[10:11, :], p["v2l"][:])
    rr.dma(m[11:13, :], ones64[:])
    return m


def _build():
    nc = bacc.Bacc("TRN2", target_bir_lowering=False, debug=False)
    x_d = nc.dram_tensor("x", [C, N], F32, kind="ExternalInput").ap()
    y_d = nc.dram_tensor("y", [C, N], F32, kind="ExternalInput").ap()
    out_d = nc.dram_tensor("o", [128, 2], F32, kind="ExternalOutput").ap()

    with tile.TileContext(nc) as tc:
        with (
            tc.tile_pool(name="prep", bufs=1) as prep,
            tc.tile_pool(name="mats", bufs=1) as mats,
            tc.tile_pool(name="parts", bufs=1) as parts,
            tc.tile_pool(name="cs", bufs=4) as csp,
            tc.tile_pool(name="mp", bufs=3) as mpp,
            tc.tile_pool(name="qq", bufs=2) as qqp,
            tc.tile_pool(name="rs", bufs=2) as rsp,
            tc.tile_pool(name="psum", bufs=2, space="PSUM") as psum,
        ):
            rr = _DmaRR(nc)
            px = _prep_pointset(nc, tc, prep, rr, x_d)
            py = _prep_pointset(nc, tc, prep, rr, y_d)
            ones64 = prep.tile([64, 128], BF16)
            nc.gpsimd.memset(ones64[:], 1.0)
            ident = prep.tile([128, 128], BF16)
            masks.make_identity(nc, ident[:])

            def replicate(m):
                for t in range(1, 4):
                    rr.dma(m[32 * t : 32 * t + K, :], m[0:K, :])

            LX = _assemble_lhs(nc, rr, mats, px, ones64, "LX")
            RY = _assemble_rhs(nc, rr, mats, py, ones64, "RY")
            replicate(LX)
            replicate(RY)

            rm = parts.tile([128, 32], F32, name="rm")
            # two independent column-min accumulators so the first half's
            # transpose+partition-reduce overlaps the second half's slabs
            CMs = [
                parts.tile([128, 4096], BF16, name="CMa"),
                parts.tile([128, 4096], BF16, name="CMb"),
            ]

            def fill_unit(r, h):
                p = psum.tile([128, 2048], F32, name="pp")
                for j in range(4):
                    nc.tensor.matmul(
                        p[:, 512 * j : 512 * (j + 1)],
                        LX[32 * j : 32 * j + K, 128 * r : 128 * (r + 1)],
                        RY[32 * j : 32 * j + K,
                           2048 * h + 512 * j : 2048 * h + 512 * (j + 1)],
                        start=True,
                        stop=True,
                        tile_position=(32 * j, 0),
                    )
                return p

            state = {"slab8_base": 0, "mslot": 0}

            def tree_flush_mpair():
                mp = state.pop("mpair")
                j0 = state["qfill"]
                q = state["q"]
                nc.vector.tensor_tensor(
                    out=q[:, j0 : j0 + 2, :],
                    in0=mp[:, :, 0:1024],
                    in1=mp[:, :, 1024:2048],
                    op=MIN,
                )
                state["qfill"] = j0 + 2
                if state["qfill"] == 8:
                    state["qfill"] = 0
                    qq = state.pop("q")
                    r8 = rsp.tile([128, 8, 512], BF16, name="r8")
                    nc.vector.tensor_tensor(
                        out=r8[:], in0=qq[:, :, 0:512], in1=qq[:, :, 512:1024],
                        op=MIN,
                    )
                    s8 = rsp.tile([128, 8, 256], BF16, name="s8")
                    nc.vector.tensor_tensor(
                        out=s8[:], in0=r8[:, :, 0:256], in1=r8[:, :, 256:512],
                        op=MIN,
                    )
                    r0 = state["slab8_base"]
                    nc.vector.tensor_reduce(
                        rm[:, r0 : r0 + 8],
                        s8[:],
                        axis=mybir.AxisListType.X,
                        op=MIN,
                    )
                    state["slab8_base"] = r0 + 8

            prev_cs = [None]  # for the r==1 bootstrap of CM

            def emit_slab(r):
                cs = csp.tile([128, 4096], BF16, name="cs")
                u0 = fill_unit(r, 0)
                nc.scalar.copy(cs[:, 0:2048], u0[:])
                u1 = fill_unit(r, 1)
                nc.scalar.copy(cs[:, 2048:4096], u1[:])

                # row direction: fold halves into the shared M pair tile
                if "mpair" not in state:
                    state["mpair"] = mpp.tile([128, 2, 2048], BF16, name="mpair")
                    state["mslot"] = 0
                if "q" not in state:
                    state["q"] = qqp.tile([128, 8, 1024], BF16, name="q")
                    state["qfill"] = 0
                mp, s = state["mpair"], state["mslot"]
                state["mslot"] = s + 1
                nc.vector.tensor_tensor(
                    out=mp[:, s, :], in0=cs[:, 0:2048], in1=cs[:, 2048:4096], op=MIN
                )
                if state["mslot"] == 2:
                    state["mslot"] = 0
                    tree_flush_mpair()

                # column direction: running elementwise min across slabs,
                # accumulated separately for slabs 0-15 and 16-31
                CM = CMs[r // 16]
                rr16 = r % 16
                if rr16 == 0:
                    prev_cs[0] = cs
                elif rr16 == 1:
                    nc.vector.tensor_tensor(
                        out=CM[:], in0=prev_cs[0][:], in1=cs[:], op=MIN
                    )
                    prev_cs[0] = None
                else:
                    nc.vector.tensor_tensor(out=CM[:], in0=CM[:], in1=cs[:], op=MIN)

            # d2 finisher for one CM accumulator: transpose 32 128x128
            # blocks into PSUM and min-reduce the transposed segments.
            def finish_cm(CM, cmname):
                cmt = parts.tile([128, 32], F32, name=cmname)
                for half in range(2):
                    pt = psum.tile([128, 2048], BF16, name="pp")
                    for j in range(16):
                        blk = 2048 * half + 128 * j
                        nc.tensor.transpose(
                            pt[:, 128 * j : 128 * (j + 1)],
                            CM[:, blk : blk + 128],
                            ident[:],
                        )
                    nc.vector.tensor_reduce(
                        cmt[:, 16 * half : 16 * half + 16],
                        pt[:].rearrange("p (s i) -> p s i", i=128),
                        axis=mybir.AxisListType.X,
                        op=MIN,
                    )
                return cmt

            for r in range(16):
                emit_slab(r)
            cma = finish_cm(CMs[0], "cma32")
            for r in range(16, NO):
                emit_slab(r)
            cmb = finish_cm(CMs[1], "cmb32")

            cm32 = parts.tile([128, 32], F32, name="cm32")
            nc.vector.tensor_tensor(out=cm32[:], in0=cma[:], in1=cmb[:], op=MIN)

            osb = parts.tile([128, 2], F32)
            nc.vector.tensor_scalar_max(rm[:], rm[:], 0.0)
            nc.vector.reduce_sum(osb[:, 0:1], rm[:], axis=mybir.AxisListType.X)
            nc.vector.tensor_scalar_max(cm32[:], cm32[:], 0.0)
            nc.vector.reduce_sum(osb[:, 1:2], cm32[:], axis=mybir.AxisListType.X)
            nc.sync.dma_start(out_d[:], osb[:])

    nc.compile()
    return nc


def kernel(ori_pcs: np.ndarray, adv_pcs: np.ndarray) -> np.ndarray:
    if "nc" not in _CACHE:
        _CACHE["nc"] = _build()
    nc = _CACHE["nc"]

    ori = np.ascontiguousarray(np.asarray(ori_pcs, dtype=np.float32))
    adv = np.ascontiguousarray(np.asarray(adv_pcs, dtype=np.float32))
    in_maps = [{"x": ori[b], "y": adv[b]} for b in range(B)]
    res = bass_utils.run_bass_kernel_spmd(nc, in_maps, core_ids=list(range(NCORES)))

    vals = []
    for b in range(B):
        o = res.results[b]["o"].astype(np.float64)
        d1 = o[:, 0].sum() / N
        d2 = o[:, 1].sum() / N
        vals.append(max(d1, d2))
    return np.array(np.mean(vals), dtype=np.float32)


# revision 13
# speedup vs baseline: 1.0308x; 1.0308x over previous
"""Trainium2 Bass kernel for ChamferLoss (B=8, C=3, N=4096), 8 NeuronCores.

Strategy: data-parallel over batch. Core b computes batch b fully:
  D[n,m] = ||x_n||^2 + ||y_m||^2 - 2 x_n.y_m   (x = ori, y = adv points)
  d1 = mean_n relu(min_m D),  d2 = mean_m relu(min_n D)
Host combines: mean_b max(d1_b, d2_b).

The -2*x.y matmul has contraction K=3; fp32 matmul is 4x slower on PE, so
each fp32 value v is split v = vh + vl (bf16 pair) and the product uses the
3-term expansion  x.y ~= xh.yh + xh.yl + xl.yh  (error ~2^-16 relative).
The squared norms are folded into the same matmul via constant-one rows, so
PSUM holds complete distance values.

Drain design (v3, single matmul pass):
 - ACT copies every [128,2048] PSUM unit into the two halves of a
   per-slab [128,4096] bf16 SBUF tile (ACT is the only engine besides DVE
   that can read PSUM; copies keep the PE->ACT pipeline free of DVE).
 - DVE works purely on bf16 SBUF at 2x rate: per slab one fold
   M = min(half0, half1) feeding a level-batched min-tree -> row minima
   (d1), plus one running accumulate CM = min(CM, Cslab) which preserves
   per-column minima across slabs.
 - Column direction (d2) finishes with PE transposes of CM (32 128x128
   blocks into PSUM) and two segmented tensor_reduce ops.
This reads each distance exactly once from PSUM and exactly twice in
bf16, with no second transposed matmul pass.
"""

import sys

sys.path.insert(0, "/opt/trn_rl_repo")

import numpy as np

import concourse.bass as bass  # noqa: F401  (registers engine types)
import concourse.masks as masks
import concourse.tile as tile
from concourse import bacc, bass_utils, mybir

B, C, N = 8, 3, 4096
NCORES = 8
NO = 32  # n_outer blocks of 128
NI = 128  # n_inner
F32 = mybir.dt.float32
BF16 = mybir.dt.bfloat16
MIN = mybir.AluOpType.min
K = 13  # contraction rows: 9 coord product terms + 2 sq rows + 2 one rows

_CACHE = {}


def _prep_pointset(nc, tc, sb, rr, v_dram):
    """Load [3, 4096] fp32 points; return dict of packed SBUF tiles."""
    vp = sb.tile([96, 128], F32)
    nc.sync.dma_start(vp[:], v_dram.rearrange("c (no ni) -> (c no) ni", ni=NI))

    vh = sb.tile([96, 128], BF16)
    nc.vector.tensor_copy(vh[:], vp[:])
    vl = sb.tile([96, 128], BF16)
    nc.vector.tensor_sub(vl[:], vp[:], vh[:])
    m2h = sb.tile([96, 128], BF16)
    nc.vector.tensor_scalar_mul(m2h[:], vh[:], -2.0)
    m2l = sb.tile([96, 128], BF16)
    nc.vector.tensor_scalar_mul(m2l[:], vl[:], -2.0)

    vsq = sb.tile([96, 128], F32)
    nc.vector.tensor_mul(vsq[:], vp[:], vp[:])
    vsqr = sb.tile([32, 384], F32)
    for c in range(3):
        rr.dma(vsqr[:, 128 * c : 128 * (c + 1)], vsq[32 * c : 32 * (c + 1), :])
    v2 = sb.tile([32, 128], F32)
    nc.vector.tensor_add(v2[:], vsqr[:, 0:128], vsqr[:, 128:256])
    nc.vector.tensor_add(v2[:], v2[:], vsqr[:, 256:384])
    v2h = sb.tile([32, 128], BF16)
    nc.vector.tensor_copy(v2h[:], v2[:])
    v2l = sb.tile([32, 128], BF16)
    nc.vector.tensor_sub(v2l[:], v2[:], v2h[:])
    return dict(vh=vh, vl=vl, m2h=m2h, m2l=m2l, v2h=v2h, v2l=v2l)


class _DmaRR:
    def __init__(self, nc):
        self.engines = [nc.sync, nc.scalar, nc.gpsimd]
        self.i = 0

    def dma(self, out, in_):
        e = self.engines[self.i % len(self.engines)]
        self.i += 1
        e.dma_start(out, in_)


def _assemble_lhs(nc, rr, sb, p, ones64, name):
    m = sb.tile([128, N], BF16, name=name)
    rr.dma(m[0:3, :], p["m2h"][:])
    rr.dma(m[3:6, :], p["m2h"][:])
    rr.dma(m[6:9, :], p["m2l"][:])
    rr.dma(m[9:11, :], ones64[:])
    rr.dma(m[11:12, :], p["v2h"][:])
    rr.dma(m[12:13, :], p["v2l"][:])
    return m


def _assemble_rhs(nc, rr, sb, p, ones64, name):
    m = sb.tile([128, N], BF16, name=name)
    rr.dma(m[0:3, :], p["vh"][:])
    rr.dma(m[3:6, :], p["vl"][:])
    rr.dma(m[6:9, :], p["vh"][:])
    rr.dma(m[9:10, :], p["v2h"][:])
    rr.dma(m[10:11, :], p["v2l"][:])
    rr.dma(m[11:13, :], ones64[:])
    return m


def _build():
    nc = bacc.Bacc("TRN2", target_bir_lowering=False, debug=False)
    x_d = nc.dram_tensor("x", [C, N], F32, kind="ExternalInput").ap()
    y_d = nc.dram_tensor("y", [C, N], F32, kind="ExternalInput").ap()
    out_d = nc.dram_tensor("o", [128, 2], F32, kind="ExternalOutput").ap()

    with tile.TileContext(nc) as tc:
        with (
            tc.tile_pool(name="prep", bufs=1) as prep,
            tc.tile_pool(name="mats", bufs=1) as mats,
            tc.tile_pool(name="parts", bufs=1) as parts,
            tc.tile_pool(name="cs", bufs=4) as csp,
            tc.tile_pool(name="mp", bufs=2) as mpp,
            tc.tile_pool(name="qq", bufs=2) as qqp,
            tc.tile_pool(name="rs", bufs=2) as rsp,
            tc.tile_pool(name="psum", bufs=2, space="PSUM") as psum,
        ):
            rr = _DmaRR(nc)
            px = _prep_pointset(nc, tc, prep, rr, x_d)
            py = _prep_pointset(nc, tc, prep, rr, y_d)
            ones64 = prep.tile([64, 128], BF16)
            nc.gpsimd.memset(ones64[:], 1.0)
            ident = prep.tile([128, 128], BF16)
            masks.make_identity(nc, ident[:])

            LX = _assemble_lhs(nc, rr, mats, px, ones64, "LX")
            RY = _assemble_rhs(nc, rr, mats, py, ones64, "RY")
            # Replicas at partition offsets 32/64/96 for the four PE
            # row-groups. Prioritize what slab 0 needs: LX cols 0:256 and
            # RY's h=0 half first, so the first fills start while the
            # bulkier replica DMAs are still in flight.
            for t in range(1, 4):
                rr.dma(LX[32 * t : 32 * t + K, 0:256], LX[0:K, 0:256])
            for t in range(1, 4):
                rr.dma(RY[32 * t : 32 * t + K, 0:2048], RY[0:K, 0:2048])
            for t in range(1, 4):
                rr.dma(LX[32 * t : 32 * t + K, 256:N], LX[0:K, 256:N])
            for t in range(1, 4):
                rr.dma(RY[32 * t : 32 * t + K, 2048:N], RY[0:K, 2048:N])

            rm = parts.tile([128, 32], F32, name="rm")
            CM = parts.tile([128, 4096], BF16, name="CM")

            def fill_unit(r, h):
                p = psum.tile([128, 2048], F32, name="pp")
                for j in range(4):
                    nc.tensor.matmul(
                        p[:, 512 * j : 512 * (j + 1)],
                        LX[32 * j : 32 * j + K, 128 * r : 128 * (r + 1)],
                        RY[32 * j : 32 * j + K,
                           2048 * h + 512 * j : 2048 * h + 512 * (j + 1)],
                        start=True,
                        stop=True,
                        tile_position=(32 * j, 0),
                    )
                return p

            state = {"slab8_base": 0, "mslot": 0}

            def tree_flush_mpair():
                mp = state.pop("mpair")
                j0 = state["qfill"]
                q = state["q"]
                nc.vector.tensor_tensor(
                    out=q[:, j0 : j0 + 4, :],
                    in0=mp[:, :, 0:1024],
                    in1=mp[:, :, 1024:2048],
                    op=MIN,
                )
                state["qfill"] = j0 + 4
                if state["qfill"] == 8:
                    state["qfill"] = 0
                    qq = state.pop("q")
                    r8 = rsp.tile([128, 8, 512], BF16, name="r8")
                    nc.vector.tensor_tensor(
                        out=r8[:], in0=qq[:, :, 0:512], in1=qq[:, :, 512:1024],
                        op=MIN,
                    )
                    s8 = rsp.tile([128, 8, 256], BF16, name="s8")
                    nc.vector.tensor_tensor(
                        out=s8[:], in0=r8[:, :, 0:256], in1=r8[:, :, 256:512],
                        op=MIN,
                    )
                    r0 = state["slab8_base"]
                    nc.vector.tensor_reduce(
                        rm[:, r0 : r0 + 8],
                        s8[:],
                        axis=mybir.AxisListType.X,
                        op=MIN,
                    )
                    state["slab8_base"] = r0 + 8

            prev_cs = [None]  # for the r==1 bootstrap of CM

            def emit_slab(r):
                cs = csp.tile([128, 4096], BF16, name="cs")
                u0 = fill_unit(r, 0)
                nc.scalar.copy(cs[:, 0:2048], u0[:])
                u1 = fill_unit(r, 1)
                nc.scalar.copy(cs[:, 2048:4096], u1[:])

                # row direction: fold halves into the shared M pair tile
                if "mpair" not in state:
                    state["mpair"] = mpp.tile([128, 4, 2048], BF16, name="mpair")
                    state["mslot"] = 0
                if "q" not in state:
                    state["q"] = qqp.tile([128, 8, 1024], BF16, name="q")
                    state["qfill"] = 0
                mp, s = state["mpair"], state["mslot"]
                state["mslot"] = s + 1
                nc.vector.tensor_tensor(
                    out=mp[:, s, :], in0=cs[:, 0:2048], in1=cs[:, 2048:4096], op=MIN
                )
                if state["mslot"] == 4:
                    state["mslot"] = 0
                    tree_flush_mpair()

                # column direction: running elementwise min across slabs
                if r == 0:
                    prev_cs[0] = cs
                elif r == 1:
                    nc.vector.tensor_tensor(
                        out=CM[:], in0=prev_cs[0][:], in1=cs[:], op=MIN
                    )
                    prev_cs[0] = None
                else:
                    nc.vector.tensor_tensor(out=CM[:], in0=CM[:], in1=cs[:], op=MIN)

            for r in range(NO):
                emit_slab(r)

            # ---- d2: transpose CM and reduce across partitions ----
            cm32 = parts.tile([128, 32], F32, name="cm32")
            for half in range(2):
                pt = psum.tile([128, 2048], BF16, name="pp")
                for j in range(16):
                    blk = 2048 * half + 128 * j
                    nc.tensor.transpose(
                        pt[:, 128 * j : 128 * (j + 1)],
                        CM[:, blk : blk + 128],
                        ident[:],
                    )
                nc.vector.tensor_reduce(
                    cm32[:, 16 * half : 16 * half + 16],
                    pt[:].rearrange("p (s i) -> p s i", i=128),
                    axis=mybir.AxisListType.X,
                    op=MIN,
                )

            osb = parts.tile([128, 2], F32)
            nc.vector.tensor_scalar_max(rm[:], rm[:], 0.0)
            nc.vector.reduce_sum(osb[:, 0:1], rm[:], axis=mybir.AxisListType.X)
            nc.vector.tensor_scalar_max(cm32[:], cm32[:], 0.0)
            nc.vector.reduce_sum(osb[:, 1:2], cm32[:], axis=mybir.AxisListType.X)
            nc.sync.dma_start(out_d[:], osb[:])

    nc.compile()
    return nc


def kernel(ori_pcs: np.ndarray, adv_pcs: np.ndarray) -> np.ndarray:
    if "nc" not in _CACHE:
        _CACHE["nc"] = _build()
    nc = _CACHE["nc"]

    ori = np.ascontiguousarray(np.asarray(ori_pcs, dtype=np.float32))
    adv = np.ascontiguousarray(np.asarray(adv_pcs, dtype=np.float32))
    in_maps = [{"x": ori[b], "y": adv[b]} for b in range(B)]
    res = bass_utils.run_bass_kernel_spmd(nc, in_maps, core_ids=list(range(NCORES)))

    vals = []
    for b in range(B):
        o = res.results[b]["o"].astype(np.float64)
        d1 = o[:, 0].sum() / N
        d2 = o[:, 1].sum() / N
        vals.append(max(d1, d2))
    return np.array(np.mean(vals), dtype=np.float32)


# revision 18
# speedup vs baseline: 1.0531x; 1.0216x over previous
"""Trainium2 Bass kernel for ChamferLoss (B=8, C=3, N=4096), 8 NeuronCores.

Strategy: data-parallel over batch. Core b computes batch b fully:
  D[n,m] = ||x_n||^2 + ||y_m||^2 - 2 x_n.y_m   (x = ori, y = adv points)
  d1 = mean_n relu(min_m D),  d2 = mean_m relu(min_n D)
Host combines: mean_b max(d1_b, d2_b).

The -2*x.y matmul has contraction K=3; fp32 matmul is 4x slower on PE, so
each fp32 value v is split v = vh + vl (bf16 pair) and the product uses the
3-term expansion  x.y ~= xh.yh + xh.yl + xl.yh  (error ~2^-16 relative).
The squared norms are folded into the same matmul via constant-one rows, so
PSUM holds complete distance values.

Drain design (v3, single matmul pass):
 - ACT copies every [128,2048] PSUM unit into the two halves of a
   per-slab [128,4096] bf16 SBUF tile (ACT is the only engine besides DVE
   that can read PSUM; copies keep the PE->ACT pipeline free of DVE).
 - DVE works purely on bf16 SBUF at 2x rate: per slab one fold
   M = min(half0, half1) feeding a level-batched min-tree -> row minima
   (d1), plus one running accumulate CM = min(CM, Cslab) which preserves
   per-column minima across slabs.
 - Column direction (d2) finishes with PE transposes of CM (32 128x128
   blocks into PSUM) and two segmented tensor_reduce ops.
This reads each distance exactly once from PSUM and exactly twice in
bf16, with no second transposed matmul pass.
"""

import sys

sys.path.insert(0, "/opt/trn_rl_repo")

import numpy as np

import concourse.bass as bass  # noqa: F401  (registers engine types)
import concourse.masks as masks
import concourse.tile as tile
from concourse import bacc, bass_utils, mybir

B, C, N = 8, 3, 4096
NCORES = 8
NO = 32  # n_outer blocks of 128
NI = 128  # n_inner
F32 = mybir.dt.float32
BF16 = mybir.dt.bfloat16
MIN = mybir.AluOpType.min
K = 13  # contraction rows: 9 coord product terms + 2 sq rows + 2 one rows

_CACHE = {}


def _prep_pointset(nc, tc, sb, rr, v_dram):
    """Load [3, 4096] fp32 points; return dict of packed SBUF tiles."""
    vp = sb.tile([96, 128], F32)
    nc.sync.dma_start(vp[:], v_dram.rearrange("c (no ni) -> (c no) ni", ni=NI))

    vh = sb.tile([96, 128], BF16)
    nc.vector.tensor_copy(vh[:], vp[:])
    vl = sb.tile([96, 128], BF16)
    nc.vector.tensor_sub(vl[:], vp[:], vh[:])
    m2h = sb.tile([96, 128], BF16)
    nc.vector.tensor_scalar_mul(m2h[:], vh[:], -2.0)
    m2l = sb.tile([96, 128], BF16)
    nc.vector.tensor_scalar_mul(m2l[:], vl[:], -2.0)

    # second load of the same points, regrouped so the 3 coordinate blocks
    # sit side by side per partition (avoids an SBUF->SBUF DMA round-trip)
    vps = sb.tile([32, 384], F32)
    nc.sync.dma_start(
        vps[:].rearrange("p (c ni) -> p c ni", ni=NI),
        v_dram.rearrange("c (no ni) -> no c ni", ni=NI),
    )
    vsqr = sb.tile([32, 384], F32)
    nc.vector.tensor_mul(vsqr[:], vps[:], vps[:])
    v2 = sb.tile([32, 128], F32)
    nc.vector.tensor_add(v2[:], vsqr[:, 0:128], vsqr[:, 128:256])
    nc.vector.tensor_add(v2[:], v2[:], vsqr[:, 256:384])
    v2h = sb.tile([32, 128], BF16)
    nc.vector.tensor_copy(v2h[:], v2[:])
    v2l = sb.tile([32, 128], BF16)
    nc.vector.tensor_sub(v2l[:], v2[:], v2h[:])
    return dict(vh=vh, vl=vl, m2h=m2h, m2l=m2l, v2h=v2h, v2l=v2l)


class _DmaRR:
    def __init__(self, nc):
        self.engines = [nc.sync, nc.scalar, nc.gpsimd]
        self.i = 0

    def dma(self, out, in_):
        e = self.engines[self.i % len(self.engines)]
        self.i += 1
        e.dma_start(out, in_)


def _assemble_lhs(nc, rr, sb, p, ones64, name):
    m = sb.tile([128, N], BF16, name=name)
    rr.dma(m[0:3, :], p["m2h"][:])
    rr.dma(m[3:6, :], p["m2h"][:])
    rr.dma(m[6:9, :], p["m2l"][:])
    rr.dma(m[9:11, :], ones64[:])
    rr.dma(m[11:12, :], p["v2h"][:])
    rr.dma(m[12:13, :], p["v2l"][:])
    return m


def _assemble_rhs(nc, rr, sb, p, ones64, name):
    m = sb.tile([128, N], BF16, name=name)
    rr.dma(m[0:3, :], p["vh"][:])
    rr.dma(m[3:6, :], p["vl"][:])
    rr.dma(m[6:9, :], p["vh"][:])
    rr.dma(m[9:10, :], p["v2h"][:])
    rr.dma(m[10:11, :], p["v2l"][:])
    rr.dma(m[11:13, :], ones64[:])
    return m


def _build():
    nc = bacc.Bacc("TRN2", target_bir_lowering=False, debug=False)
    x_d = nc.dram_tensor("x", [C, N], F32, kind="ExternalInput").ap()
    y_d = nc.dram_tensor("y", [C, N], F32, kind="ExternalInput").ap()
    out_d = nc.dram_tensor("o", [128, 2], F32, kind="ExternalOutput").ap()

    with tile.TileContext(nc) as tc:
        with (
            tc.tile_pool(name="prep", bufs=1) as prep,
            tc.tile_pool(name="mats", bufs=1) as mats,
            tc.tile_pool(name="parts", bufs=1) as parts,
            tc.tile_pool(name="cs", bufs=4) as csp,
            tc.tile_pool(name="mp", bufs=2) as mpp,
            tc.tile_pool(name="qq", bufs=2) as qqp,
            tc.tile_pool(name="rs", bufs=2) as rsp,
            tc.tile_pool(name="psum", bufs=2, space="PSUM") as psum,
        ):
            rr = _DmaRR(nc)
            px = _prep_pointset(nc, tc, prep, rr, x_d)
            py = _prep_pointset(nc, tc, prep, rr, y_d)
            ones64 = prep.tile([64, 128], BF16)
            nc.gpsimd.memset(ones64[:], 1.0)
            ident = prep.tile([128, 128], BF16)
            masks.make_identity(nc, ident[:])

            LX = _assemble_lhs(nc, rr, mats, px, ones64, "LX")
            RY = _assemble_rhs(nc, rr, mats, py, ones64, "RY")
            # Replicas at partition offsets 32/64/96 for the four PE
            # row-groups. Prioritize what slab 0 needs: LX cols 0:256 and
            # RY's h=0 half first, so the first fills start while the
            # bulkier replica DMAs are still in flight.
            for t in range(1, 4):
                rr.dma(LX[32 * t : 32 * t + K, 0:256], LX[0:K, 0:256])
            for t in range(1, 4):
                rr.dma(RY[32 * t : 32 * t + K, 0:2048], RY[0:K, 0:2048])
            for t in range(1, 4):
                rr.dma(LX[32 * t : 32 * t + K, 256:N], LX[0:K, 256:N])
            for t in range(1, 4):
                rr.dma(RY[32 * t : 32 * t + K, 2048:N], RY[0:K, 2048:N])

            rm = parts.tile([128, 32], F32, name="rm")
            CM = parts.tile([128, 4096], BF16, name="CM")

            def fill_unit(r, h):
                # Slabs 0-1 run on PE row-group 0 only (no replicas needed)
                # so the drain pipeline starts before the replica DMAs land.
                quad = r >= 2
                p = psum.tile([128, 2048], F32, name="pp")
                for j in range(4):
                    g = j if quad else 0
                    nc.tensor.matmul(
                        p[:, 512 * j : 512 * (j + 1)],
                        LX[32 * g : 32 * g + K, 128 * r : 128 * (r + 1)],
                        RY[32 * g : 32 * g + K,
                           2048 * h + 512 * j : 2048 * h + 512 * (j + 1)],
                        start=True,
                        stop=True,
                        tile_position=(32 * g, 0),
                    )
                return p

            state = {"slab8_base": 0, "mslot": 0}

            def tree_flush_mpair():
                mp = state.pop("mpair")
                j0 = state["qfill"]
                q = state["q"]
                nc.vector.tensor_tensor(
                    out=q[:, j0 : j0 + 4, :],
                    in0=mp[:, :, 0:1024],
                    in1=mp[:, :, 1024:2048],
                    op=MIN,
                )
                state["qfill"] = j0 + 4
                if state["qfill"] == 8:
                    state["qfill"] = 0
                    qq = state.pop("q")
                    r8 = rsp.tile([128, 8, 512], BF16, name="r8")
                    nc.vector.tensor_tensor(
                        out=r8[:], in0=qq[:, :, 0:512], in1=qq[:, :, 512:1024],
                        op=MIN,
                    )
                    s8 = rsp.tile([128, 8, 256], BF16, name="s8")
                    nc.vector.tensor_tensor(
                        out=s8[:], in0=r8[:, :, 0:256], in1=r8[:, :, 256:512],
                        op=MIN,
                    )
                    r0 = state["slab8_base"]
                    nc.vector.tensor_reduce(
                        rm[:, r0 : r0 + 8],
                        s8[:],
                        axis=mybir.AxisListType.X,
                        op=MIN,
                    )
                    state["slab8_base"] = r0 + 8

            prev_cs = [None]  # for the r==1 bootstrap of CM

            def emit_slab(r):
                cs = csp.tile([128, 4096], BF16, name="cs")
                u0 = fill_unit(r, 0)
                nc.scalar.copy(cs[:, 0:2048], u0[:])
                u1 = fill_unit(r, 1)
                nc.scalar.copy(cs[:, 2048:4096], u1[:])

                # row direction: fold halves into the shared M pair tile
                if "mpair" not in state:
                    state["mpair"] = mpp.tile([128, 4, 2048], BF16, name="mpair")
                    state["mslot"] = 0
                if "q" not in state:
                    state["q"] = qqp.tile([128, 8, 1024], BF16, name="q")
                    state["qfill"] = 0
                mp, s = state["mpair"], state["mslot"]
                state["mslot"] = s + 1
                nc.vector.tensor_tensor(
                    out=mp[:, s, :], in0=cs[:, 0:2048], in1=cs[:, 2048:4096], op=MIN
                )
                if state["mslot"] == 4:
                    state["mslot"] = 0
                    tree_flush_mpair()

                # column direction: running elementwise min across slabs
                if r == 0:
                    prev_cs[0] = cs
                elif r == 1:
                    nc.vector.tensor_tensor(
                        out=CM[:], in0=prev_cs[0][:], in1=cs[:], op=MIN
                    )
                    prev_cs[0] = None
                else:
                    nc.vector.tensor_tensor(out=CM[:], in0=CM[:], in1=cs[:], op=MIN)

            for r in range(NO):
                emit_slab(r)

            # ---- d2: transpose CM and reduce across partitions ----
            cm32 = parts.tile([128, 32], F32, name="cm32")
            for half in range(2):
                pt = psum.tile([128, 2048], BF16, name="pp")
                for j in range(16):
                    blk = 2048 * half + 128 * j
                    nc.tensor.transpose(
                        pt[:, 128 * j : 128 * (j + 1)],
                        CM[:, blk : blk + 128],
                        ident[:],
                    )
                nc.vector.tensor_reduce(
                    cm32[:, 16 * half : 16 * half + 16],
                    pt[:].rearrange("p (s i) -> p s i", i=128),
                    axis=mybir.AxisListType.X,
                    op=MIN,
                )

            osb = parts.tile([128, 2], F32)
            nc.vector.tensor_scalar_max(rm[:], rm[:], 0.0)
            nc.vector.reduce_sum(osb[:, 0:1], rm[:], axis=mybir.AxisListType.X)
            nc.vector.tensor_scalar_max(cm32[:], cm32[:], 0.0)
            nc.vector.reduce_sum(osb[:, 1:2], cm32[:], axis=mybir.AxisListType.X)
            nc.sync.dma_start(out_d[:], osb[:])

    nc.compile()
    return nc


def kernel(ori_pcs: np.ndarray, adv_pcs: np.ndarray) -> np.ndarray:
    if "nc" not in _CACHE:
        _CACHE["nc"] = _build()
    nc = _CACHE["nc"]

    ori = np.ascontiguousarray(np.asarray(ori_pcs, dtype=np.float32))
    adv = np.ascontiguousarray(np.asarray(adv_pcs, dtype=np.float32))
    in_maps = [{"x": ori[b], "y": adv[b]} for b in range(B)]
    res = bass_utils.run_bass_kernel_spmd(nc, in_maps, core_ids=list(range(NCORES)))

    vals = []
    for b in range(B):
        o = res.results[b]["o"].astype(np.float64)
        d1 = o[:, 0].sum() / N
        d2 = o[:, 1].sum() / N
        vals.append(max(d1, d2))
    return np.array(np.mean(vals), dtype=np.float32)


# revision 19
# speedup vs baseline: 1.0635x; 1.0098x over previous
"""Trainium2 Bass kernel for ChamferLoss (B=8, C=3, N=4096), 8 NeuronCores.

Strategy: data-parallel over batch. Core b computes batch b fully:
  D[n,m] = ||x_n||^2 + ||y_m||^2 - 2 x_n.y_m   (x = ori, y = adv points)
  d1 = mean_n relu(min_m D),  d2 = mean_m relu(min_n D)
Host combines: mean_b max(d1_b, d2_b).

The -2*x.y matmul has contraction K=3; fp32 matmul is 4x slower on PE, so
each fp32 value v is split v = vh + vl (bf16 pair) and the product uses the
3-term expansion  x.y ~= xh.yh + xh.yl + xl.yh  (error ~2^-16 relative).
The squared norms are folded into the same matmul via constant-one rows, so
PSUM holds complete distance values.

Drain design (v3, single matmul pass):
 - ACT copies every [128,2048] PSUM unit into the two halves of a
   per-slab [128,4096] bf16 SBUF tile (ACT is the only engine besides DVE
   that can read PSUM; copies keep the PE->ACT pipeline free of DVE).
 - DVE works purely on bf16 SBUF at 2x rate: per slab one fold
   M = min(half0, half1) feeding a level-batched min-tree -> row minima
   (d1), plus one running accumulate CM = min(CM, Cslab) which preserves
   per-column minima across slabs.
 - Column direction (d2) finishes with PE transposes of CM (32 128x128
   blocks into PSUM) and two segmented tensor_reduce ops.
This reads each distance exactly once from PSUM and exactly twice in
bf16, with no second transposed matmul pass.
"""

import sys

sys.path.insert(0, "/opt/trn_rl_repo")

import numpy as np

import concourse.bass as bass  # noqa: F401  (registers engine types)
import concourse.masks as masks
import concourse.tile as tile
from concourse import bacc, bass_utils, mybir

B, C, N = 8, 3, 4096
NCORES = 8
NO = 32  # n_outer blocks of 128
NI = 128  # n_inner
F32 = mybir.dt.float32
BF16 = mybir.dt.bfloat16
MIN = mybir.AluOpType.min
K = 13  # contraction rows: 9 coord product terms + 2 sq rows + 2 one rows

_CACHE = {}


def _prep_pointset(nc, tc, sb, rr, v_dram):
    """Load [3, 4096] fp32 points; return dict of packed SBUF tiles."""
    vp = sb.tile([96, 128], F32)
    nc.sync.dma_start(vp[:], v_dram.rearrange("c (no ni) -> (c no) ni", ni=NI))

    vh = sb.tile([96, 128], BF16)
    nc.vector.tensor_copy(vh[:], vp[:])
    vl = sb.tile([96, 128], BF16)
    nc.vector.tensor_sub(vl[:], vp[:], vh[:])
    m2h = sb.tile([96, 128], BF16)
    nc.vector.tensor_scalar_mul(m2h[:], vh[:], -2.0)
    m2l = sb.tile([96, 128], BF16)
    nc.vector.tensor_scalar_mul(m2l[:], vl[:], -2.0)

    # second load of the same points, regrouped so the 3 coordinate blocks
    # sit side by side per partition (avoids an SBUF->SBUF DMA round-trip)
    vps = sb.tile([32, 384], F32)
    nc.sync.dma_start(
        vps[:].rearrange("p (c ni) -> p c ni", ni=NI),
        v_dram.rearrange("c (no ni) -> no c ni", ni=NI),
    )
    vsqr = sb.tile([32, 384], F32)
    nc.vector.tensor_mul(vsqr[:], vps[:], vps[:])
    v2 = sb.tile([32, 128], F32)
    nc.vector.tensor_add(v2[:], vsqr[:, 0:128], vsqr[:, 128:256])
    nc.vector.tensor_add(v2[:], v2[:], vsqr[:, 256:384])
    v2h = sb.tile([32, 128], BF16)
    nc.vector.tensor_copy(v2h[:], v2[:])
    v2l = sb.tile([32, 128], BF16)
    nc.vector.tensor_sub(v2l[:], v2[:], v2h[:])
    return dict(vh=vh, vl=vl, m2h=m2h, m2l=m2l, v2h=v2h, v2l=v2l)


class _DmaRR:
    def __init__(self, nc):
        self.engines = [nc.sync, nc.scalar, nc.gpsimd]
        self.i = 0

    def dma(self, out, in_):
        e = self.engines[self.i % len(self.engines)]
        self.i += 1
        e.dma_start(out, in_)


def _assemble_lhs(nc, rr, sb, p, ones64, name):
    m = sb.tile([128, N], BF16, name=name)
    rr.dma(m[0:3, :], p["m2h"][:])
    rr.dma(m[3:6, :], p["m2h"][:])
    rr.dma(m[6:9, :], p["m2l"][:])
    rr.dma(m[9:11, :], ones64[:])
    rr.dma(m[11:12, :], p["v2h"][:])
    rr.dma(m[12:13, :], p["v2l"][:])
    return m


def _assemble_rhs(nc, rr, sb, p, ones64, name):
    m = sb.tile([128, N], BF16, name=name)
    rr.dma(m[0:3, :], p["vh"][:])
    rr.dma(m[3:6, :], p["vl"][:])
    rr.dma(m[6:9, :], p["vh"][:])
    rr.dma(m[9:10, :], p["v2h"][:])
    rr.dma(m[10:11, :], p["v2l"][:])
    rr.dma(m[11:13, :], ones64[:])
    return m


def _build():
    nc = bacc.Bacc("TRN2", target_bir_lowering=False, debug=False)
    x_d = nc.dram_tensor("x", [C, N], F32, kind="ExternalInput").ap()
    y_d = nc.dram_tensor("y", [C, N], F32, kind="ExternalInput").ap()
    out_d = nc.dram_tensor("o", [128, 2], F32, kind="ExternalOutput").ap()

    with tile.TileContext(nc) as tc:
        with (
            tc.tile_pool(name="prep", bufs=1) as prep,
            tc.tile_pool(name="mats", bufs=1) as mats,
            tc.tile_pool(name="parts", bufs=1) as parts,
            tc.tile_pool(name="cs", bufs=6) as csp,
            tc.tile_pool(name="mp", bufs=2) as mpp,
            tc.tile_pool(name="qq", bufs=2) as qqp,
            tc.tile_pool(name="rs", bufs=2) as rsp,
            tc.tile_pool(name="psum", bufs=2, space="PSUM") as psum,
        ):
            rr = _DmaRR(nc)
            px = _prep_pointset(nc, tc, prep, rr, x_d)
            py = _prep_pointset(nc, tc, prep, rr, y_d)
            ones64 = prep.tile([64, 128], BF16)
            nc.gpsimd.memset(ones64[:], 1.0)
            ident = prep.tile([128, 128], BF16)
            masks.make_identity(nc, ident[:])

            LX = _assemble_lhs(nc, rr, mats, px, ones64, "LX")
            RY = _assemble_rhs(nc, rr, mats, py, ones64, "RY")
            # Replicas at partition offsets 32/64/96 for the four PE
            # row-groups. Prioritize what slab 0 needs: LX cols 0:256 and
            # RY's h=0 half first, so the first fills start while the
            # bulkier replica DMAs are still in flight.
            for t in range(1, 4):
                rr.dma(LX[32 * t : 32 * t + K, 0:256], LX[0:K, 0:256])
            for t in range(1, 4):
                rr.dma(RY[32 * t : 32 * t + K, 0:2048], RY[0:K, 0:2048])
            for t in range(1, 4):
                rr.dma(LX[32 * t : 32 * t + K, 256:N], LX[0:K, 256:N])
            for t in range(1, 4):
                rr.dma(RY[32 * t : 32 * t + K, 2048:N], RY[0:K, 2048:N])

            rm = parts.tile([128, 32], F32, name="rm")
            CM = parts.tile([128, 4096], BF16, name="CM")

            def fill_unit(r, h):
                # Slabs 0-1 run on PE row-group 0 only (no replicas needed)
                # so the drain pipeline starts before the replica DMAs land.
                quad = r >= 2
                p = psum.tile([128, 2048], F32, name="pp")
                for j in range(4):
                    g = j if quad else 0
                    nc.tensor.matmul(
                        p[:, 512 * j : 512 * (j + 1)],
                        LX[32 * g : 32 * g + K, 128 * r : 128 * (r + 1)],
                        RY[32 * g : 32 * g + K,
                           2048 * h + 512 * j : 2048 * h + 512 * (j + 1)],
                        start=True,
                        stop=True,
                        tile_position=(32 * g, 0),
                    )
                return p

            state = {"slab8_base": 0, "mslot": 0}

            def tree_flush_mpair():
                mp = state.pop("mpair")
                j0 = state["qfill"]
                q = state["q"]
                nc.vector.tensor_tensor(
                    out=q[:, j0 : j0 + 4, :],
                    in0=mp[:, :, 0:1024],
                    in1=mp[:, :, 1024:2048],
                    op=MIN,
                )
                state["qfill"] = j0 + 4
                if state["qfill"] == 8:
                    state["qfill"] = 0
                    qq = state.pop("q")
                    r8 = rsp.tile([128, 8, 512], BF16, name="r8")
                    nc.vector.tensor_tensor(
                        out=r8[:], in0=qq[:, :, 0:512], in1=qq[:, :, 512:1024],
                        op=MIN,
                    )
                    s8 = rsp.tile([128, 8, 256], BF16, name="s8")
                    nc.vector.tensor_tensor(
                        out=s8[:], in0=r8[:, :, 0:256], in1=r8[:, :, 256:512],
                        op=MIN,
                    )
                    r0 = state["slab8_base"]
                    nc.vector.tensor_reduce(
                        rm[:, r0 : r0 + 8],
                        s8[:],
                        axis=mybir.AxisListType.X,
                        op=MIN,
                    )
                    state["slab8_base"] = r0 + 8

            prev_cs = [None]  # for the r==1 bootstrap of CM

            def emit_slab(r):
                cs = csp.tile([128, 4096], BF16, name="cs")
                u0 = fill_unit(r, 0)
                nc.scalar.copy(cs[:, 0:2048], u0[:])
                u1 = fill_unit(r, 1)
                nc.scalar.copy(cs[:, 2048:4096], u1[:])

                # row direction: fold halves into the shared M pair tile
                if "mpair" not in state:
                    state["mpair"] = mpp.tile([128, 4, 2048], BF16, name="mpair")
                    state["mslot"] = 0
                if "q" not in state:
                    state["q"] = qqp.tile([128, 8, 1024], BF16, name="q")
                    state["qfill"] = 0
                mp, s = state["mpair"], state["mslot"]
                state["mslot"] = s + 1
                nc.vector.tensor_tensor(
                    out=mp[:, s, :], in0=cs[:, 0:2048], in1=cs[:, 2048:4096], op=MIN
                )
                if state["mslot"] == 4:
                    state["mslot"] = 0
                    tree_flush_mpair()

                # column direction: running elementwise min across slabs
                if r == 0:
                    prev_cs[0] = cs
                elif r == 1:
                    nc.vector.tensor_tensor(
                        out=CM[:], in0=prev_cs[0][:], in1=cs[:], op=MIN
                    )
                    prev_cs[0] = None
                else:
                    nc.vector.tensor_tensor(out=CM[:], in0=CM[:], in1=cs[:], op=MIN)

            for r in range(NO):
                emit_slab(r)

            # ---- d2: transpose CM and reduce across partitions ----
            cm32 = parts.tile([128, 32], F32, name="cm32")
            for half in range(2):
                pt = psum.tile([128, 2048], BF16, name="pp")
                for j in range(16):
                    blk = 2048 * half + 128 * j
                    nc.tensor.transpose(
                        pt[:, 128 * j : 128 * (j + 1)],
                        CM[:, blk : blk + 128],
                        ident[:],
                    )
                nc.vector.tensor_reduce(
                    cm32[:, 16 * half : 16 * half + 16],
                    pt[:].rearrange("p (s i) -> p s i", i=128),
                    axis=mybir.AxisListType.X,
                    op=MIN,
                )

            osb = parts.tile([128, 2], F32)
            nc.vector.tensor_scalar_max(rm[:], rm[:], 0.0)
            nc.vector.reduce_sum(osb[:, 0:1], rm[:], axis=mybir.AxisListType.X)
            nc.vector.tensor_scalar_max(cm32[:], cm32[:], 0.0)
            nc.vector.reduce_sum(osb[:, 1:2], cm32[:], axis=mybir.AxisListType.X)
            nc.sync.dma_start(out_d[:], osb[:])

    nc.compile()
    return nc


def kernel(ori_pcs: np.ndarray, adv_pcs: np.ndarray) -> np.ndarray:
    if "nc" not in _CACHE:
        _CACHE["nc"] = _build()
    nc = _CACHE["nc"]

    ori = np.ascontiguousarray(np.asarray(ori_pcs, dtype=np.float32))
    adv = np.ascontiguousarray(np.asarray(adv_pcs, dtype=np.float32))
    in_maps = [{"x": ori[b], "y": adv[b]} for b in range(B)]
    res = bass_utils.run_bass_kernel_spmd(nc, in_maps, core_ids=list(range(NCORES)))

    vals = []
    for b in range(B):
        o = res.results[b]["o"].astype(np.float64)
        d1 = o[:, 0].sum() / N
        d2 = o[:, 1].sum() / N
        vals.append(max(d1, d2))
    return np.array(np.mean(vals), dtype=np.float32)


# revision 20
# speedup vs baseline: 1.0681x; 1.0044x over previous
"""Trainium2 Bass kernel for ChamferLoss (B=8, C=3, N=4096), 8 NeuronCores.

Strategy: data-parallel over batch. Core b computes batch b fully:
  D[n,m] = ||x_n||^2 + ||y_m||^2 - 2 x_n.y_m   (x = ori, y = adv points)
  d1 = mean_n relu(min_m D),  d2 = mean_m relu(min_n D)
Host combines: mean_b max(d1_b, d2_b).

The -2*x.y matmul has contraction K=3; fp32 matmul is 4x slower on PE, so
each fp32 value v is split v = vh + vl (bf16 pair) and the product uses the
3-term expansion  x.y ~= xh.yh + xh.yl + xl.yh  (error ~2^-16 relative).
The squared norms are folded into the same matmul via constant-one rows, so
PSUM holds complete distance values.

Drain design (v3, single matmul pass):
 - ACT copies every [128,2048] PSUM unit into the two halves of a
   per-slab [128,4096] bf16 SBUF tile (ACT is the only engine besides DVE
   that can read PSUM; copies keep the PE->ACT pipeline free of DVE).
 - DVE works purely on bf16 SBUF at 2x rate: per slab one fold
   M = min(half0, half1) feeding a level-batched min-tree -> row minima
   (d1), plus one running accumulate CM = min(CM, Cslab) which preserves
   per-column minima across slabs.
 - Column direction (d2) finishes with PE transposes of CM (32 128x128
   blocks into PSUM) and two segmented tensor_reduce ops.
This reads each distance exactly once from PSUM and exactly twice in
bf16, with no second transposed matmul pass.
"""

import sys

sys.path.insert(0, "/opt/trn_rl_repo")

import numpy as np

import concourse.bass as bass  # noqa: F401  (registers engine types)
import concourse.masks as masks
import concourse.tile as tile
from concourse import bacc, bass_utils, mybir

B, C, N = 8, 3, 4096
NCORES = 8
NO = 32  # n_outer blocks of 128
NI = 128  # n_inner
F32 = mybir.dt.float32
BF16 = mybir.dt.bfloat16
MIN = mybir.AluOpType.min
K = 13  # contraction rows: 9 coord product terms + 2 sq rows + 2 one rows

_CACHE = {}


def _prep_pointset(nc, tc, sb, rr, v_dram):
    """Load [3, 4096] fp32 points; return dict of packed SBUF tiles."""
    vp = sb.tile([96, 128], F32)
    nc.sync.dma_start(vp[:], v_dram.rearrange("c (no ni) -> (c no) ni", ni=NI))

    vh = sb.tile([96, 128], BF16)
    nc.vector.tensor_copy(vh[:], vp[:])
    vl = sb.tile([96, 128], BF16)
    nc.vector.tensor_sub(vl[:], vp[:], vh[:])
    m2h = sb.tile([96, 128], BF16)
    nc.vector.tensor_scalar_mul(m2h[:], vh[:], -2.0)
    m2l = sb.tile([96, 128], BF16)
    nc.vector.tensor_scalar_mul(m2l[:], vl[:], -2.0)

    # second load of the same points, regrouped so the 3 coordinate blocks
    # sit side by side per partition (avoids an SBUF->SBUF DMA round-trip)
    vps = sb.tile([32, 384], F32)
    nc.sync.dma_start(
        vps[:].rearrange("p (c ni) -> p c ni", ni=NI),
        v_dram.rearrange("c (no ni) -> no c ni", ni=NI),
    )
    vsqr = sb.tile([32, 384], F32)
    nc.vector.tensor_mul(vsqr[:], vps[:], vps[:])
    v2 = sb.tile([32, 128], F32)
    nc.vector.tensor_add(v2[:], vsqr[:, 0:128], vsqr[:, 128:256])
    nc.vector.tensor_add(v2[:], v2[:], vsqr[:, 256:384])
    v2h = sb.tile([32, 128], BF16)
    nc.vector.tensor_copy(v2h[:], v2[:])
    v2l = sb.tile([32, 128], BF16)
    nc.vector.tensor_sub(v2l[:], v2[:], v2h[:])
    return dict(vh=vh, vl=vl, m2h=m2h, m2l=m2l, v2h=v2h, v2l=v2l)


class _DmaRR:
    def __init__(self, nc):
        self.engines = [nc.sync, nc.scalar, nc.gpsimd]
        self.i = 0

    def dma(self, out, in_):
        e = self.engines[self.i % len(self.engines)]
        self.i += 1
        e.dma_start(out, in_)


def _assemble_lhs(nc, rr, sb, p, ones64, name):
    m = sb.tile([128, N], BF16, name=name)
    rr.dma(m[0:3, :], p["m2h"][:])
    rr.dma(m[3:6, :], p["m2h"][:])
    rr.dma(m[6:9, :], p["m2l"][:])
    rr.dma(m[9:11, :], ones64[:])
    rr.dma(m[11:12, :], p["v2h"][:])
    rr.dma(m[12:13, :], p["v2l"][:])
    return m


def _assemble_rhs(nc, rr, sb, p, ones64, name):
    m = sb.tile([128, N], BF16, name=name)
    rr.dma(m[0:3, :], p["vh"][:])
    rr.dma(m[3:6, :], p["vl"][:])
    rr.dma(m[6:9, :], p["vh"][:])
    rr.dma(m[9:10, :], p["v2h"][:])
    rr.dma(m[10:11, :], p["v2l"][:])
    rr.dma(m[11:13, :], ones64[:])
    return m


def _build():
    nc = bacc.Bacc("TRN2", target_bir_lowering=False, debug=False)
    x_d = nc.dram_tensor("x", [C, N], F32, kind="ExternalInput").ap()
    y_d = nc.dram_tensor("y", [C, N], F32, kind="ExternalInput").ap()
    out_d = nc.dram_tensor("o", [128, 2], F32, kind="ExternalOutput").ap()

    with tile.TileContext(nc) as tc:
        with (
            tc.tile_pool(name="prep", bufs=1) as prep,
            tc.tile_pool(name="mats", bufs=1) as mats,
            tc.tile_pool(name="parts", bufs=1) as parts,
            tc.tile_pool(name="cs", bufs=6) as csp,
            tc.tile_pool(name="mp", bufs=3) as mpp,
            tc.tile_pool(name="qq", bufs=3) as qqp,
            tc.tile_pool(name="rs", bufs=2) as rsp,
            tc.tile_pool(name="psum", bufs=2, space="PSUM") as psum,
        ):
            rr = _DmaRR(nc)
            px = _prep_pointset(nc, tc, prep, rr, x_d)
            py = _prep_pointset(nc, tc, prep, rr, y_d)
            ones64 = prep.tile([64, 128], BF16)
            nc.gpsimd.memset(ones64[:], 1.0)
            ident = prep.tile([128, 128], BF16)
            masks.make_identity(nc, ident[:])

            LX = _assemble_lhs(nc, rr, mats, px, ones64, "LX")
            RY = _assemble_rhs(nc, rr, mats, py, ones64, "RY")
            # Replicas at partition offsets 32/64/96 for the four PE
            # row-groups. Prioritize what slab 0 needs: LX cols 0:256 and
            # RY's h=0 half first, so the first fills start while the
            # bulkier replica DMAs are still in flight.
            for t in range(1, 4):
                rr.dma(LX[32 * t : 32 * t + K, 0:256], LX[0:K, 0:256])
            for t in range(1, 4):
                rr.dma(RY[32 * t : 32 * t + K, 0:2048], RY[0:K, 0:2048])
            for t in range(1, 4):
                rr.dma(LX[32 * t : 32 * t + K, 256:N], LX[0:K, 256:N])
            for t in range(1, 4):
                rr.dma(RY[32 * t : 32 * t + K, 2048:N], RY[0:K, 2048:N])

            rm = parts.tile([128, 32], F32, name="rm")
            CM = parts.tile([128, 4096], BF16, name="CM")

            def fill_unit(r, h):
                # Slabs 0-1 run on PE row-group 0 only (no replicas needed)
                # so the drain pipeline starts before the replica DMAs land.
                quad = r >= 2
                p = psum.tile([128, 2048], F32, name="pp")
                for j in range(4):
                    g = j if quad else 0
                    nc.tensor.matmul(
                        p[:, 512 * j : 512 * (j + 1)],
                        LX[32 * g : 32 * g + K, 128 * r : 128 * (r + 1)],
                        RY[32 * g : 32 * g + K,
                           2048 * h + 512 * j : 2048 * h + 512 * (j + 1)],
                        start=True,
                        stop=True,
                        tile_position=(32 * g, 0),
                    )
                return p

            state = {"slab8_base": 0, "mslot": 0}

            def tree_flush_mpair():
                mp = state.pop("mpair")
                j0 = state["qfill"]
                q = state["q"]
                nc.vector.tensor_tensor(
                    out=q[:, j0 : j0 + 4, :],
                    in0=mp[:, :, 0:1024],
                    in1=mp[:, :, 1024:2048],
                    op=MIN,
                )
                state["qfill"] = j0 + 4
                if state["qfill"] == 8:
                    state["qfill"] = 0
                    qq = state.pop("q")
                    r8 = rsp.tile([128, 8, 512], BF16, name="r8")
                    nc.vector.tensor_tensor(
                        out=r8[:], in0=qq[:, :, 0:512], in1=qq[:, :, 512:1024],
                        op=MIN,
                    )
                    s8 = rsp.tile([128, 8, 256], BF16, name="s8")
                    nc.vector.tensor_tensor(
                        out=s8[:], in0=r8[:, :, 0:256], in1=r8[:, :, 256:512],
                        op=MIN,
                    )
                    r0 = state["slab8_base"]
                    nc.vector.tensor_reduce(
                        rm[:, r0 : r0 + 8],
                        s8[:],
                        axis=mybir.AxisListType.X,
                        op=MIN,
                    )
                    state["slab8_base"] = r0 + 8

            prev_cs = [None]  # for the r==1 bootstrap of CM

            def emit_slab(r):
                cs = csp.tile([128, 4096], BF16, name="cs")
                u0 = fill_unit(r, 0)
                nc.scalar.copy(cs[:, 0:2048], u0[:])
                u1 = fill_unit(r, 1)
                nc.scalar.copy(cs[:, 2048:4096], u1[:])

                # row direction: fold halves into the shared M pair tile
                if "mpair" not in state:
                    state["mpair"] = mpp.tile([128, 4, 2048], BF16, name="mpair")
                    state["mslot"] = 0
                if "q" not in state:
                    state["q"] = qqp.tile([128, 8, 1024], BF16, name="q")
                    state["qfill"] = 0
                mp, s = state["mpair"], state["mslot"]
                state["mslot"] = s + 1
                nc.vector.tensor_tensor(
                    out=mp[:, s, :], in0=cs[:, 0:2048], in1=cs[:, 2048:4096], op=MIN
                )
                if state["mslot"] == 4:
                    state["mslot"] = 0
                    tree_flush_mpair()

                # column direction: running elementwise min across slabs
                if r == 0:
                    prev_cs[0] = cs
                elif r == 1:
                    nc.vector.tensor_tensor(
                        out=CM[:], in0=prev_cs[0][:], in1=cs[:], op=MIN
                    )
                    prev_cs[0] = None
                else:
                    nc.vector.tensor_tensor(out=CM[:], in0=CM[:], in1=cs[:], op=MIN)

            for r in range(NO):
                emit_slab(r)

            # ---- d2: transpose CM and reduce across partitions ----
            cm32 = parts.tile([128, 32], F32, name="cm32")
            for half in range(2):
                pt = psum.tile([128, 2048], BF16, name="pp")
                for j in range(16):
                    blk = 2048 * half + 128 * j
                    nc.tensor.transpose(
                        pt[:, 128 * j : 128 * (j + 1)],
                        CM[:, blk : blk + 128],
                        ident[:],
                    )
                nc.vector.tensor_reduce(
                    cm32[:, 16 * half : 16 * half + 16],
                    pt[:].rearrange("p (s i) -> p s i", i=128),
                    axis=mybir.AxisListType.X,
                    op=MIN,
                )

            osb = parts.tile([128, 2], F32)
            nc.vector.tensor_scalar_max(rm[:], rm[:], 0.0)
            nc.vector.reduce_sum(osb[:, 0:1], rm[:], axis=mybir.AxisListType.X)
            nc.vector.tensor_scalar_max(cm32[:], cm32[:], 0.0)
            nc.vector.reduce_sum(osb[:, 1:2], cm32[:], axis=mybir.AxisListType.X)
            nc.sync.dma_start(out_d[:], osb[:])

    nc.compile()
    return nc


def kernel(ori_pcs: np.ndarray, adv_pcs: np.ndarray) -> np.ndarray:
    if "nc" not in _CACHE:
        _CACHE["nc"] = _build()
    nc = _CACHE["nc"]

    ori = np.ascontiguousarray(np.asarray(ori_pcs, dtype=np.float32))
    adv = np.ascontiguousarray(np.asarray(adv_pcs, dtype=np.float32))
    in_maps = [{"x": ori[b], "y": adv[b]} for b in range(B)]
    res = bass_utils.run_bass_kernel_spmd(nc, in_maps, core_ids=list(range(NCORES)))

    vals = []
    for b in range(B):
        o = res.results[b]["o"].astype(np.float64)
        d1 = o[:, 0].sum() / N
        d2 = o[:, 1].sum() / N
        vals.append(max(d1, d2))
    return np.array(np.mean(vals), dtype=np.float32)
